# revision 37
# baseline (speedup 1.0000x reference)
"""Trainium2 Bass kernel for nn_DRAM_MAC_temporal_encoding (polynomial attention).

Math (QK_mul=1):
    out = sum_i coef_i * (x @ (y-OFF)^i) * decay
        = (x * decay) @ P(y-OFF)            # P = Horner cubic, elementwise
so the whole problem is ONE [S,64]@[64,S] matmul per (b,h) head plus the
output write -> memory-bound. The tiny elementwise prep (poly on y,
row-scaling x, transpose, bf16 cast) runs on host; the device does
matmuls + store. QK_mul=0: out = sum_i coef_i * ((x*d^i) @ (y-OFF)^i) ->
two K=128 chunks (4 stacked K=64 terms), same kernel with n_chunks=2.

Perf model (per core: 3 heads, 25.2 MiB of output):
- Tolerance is rel_err < 2e-2, so inputs are plain bf16 (one matmul per
  output tile, no hi/lo split; measured 1.2e-3) and the output is stored
  as fp16 (half the fp32 store traffic; host upcasts). DMA is the
  bottleneck: 16 engines x ~25.6 GB/s = ~410 GB/s per core, ~63 us of
  stores, and in clean windows the engines run at 99-101% utilization.
- K=64 matmuls stream at ~1/3 the K=128 rate on HW (630 vs 233 ns per
  [128,512]), so K is presented as 128 with zero rows 64:128 (memset once;
  zero rows add no HBM traffic).
- PSUM->SBUF drains (fp32->fp16) are the only engines that can read PSUM
  (DVE + Act; Pool cannot): [128,1024] half-tiles on a 4-deep PSUM ring,
  greedily balanced across the two engines. A 2-deep [128,2048] ring
  serialized PE fill against the 2.2 us whole-tile drain.
- Startup: head 0's zero-row memsets go to DVE + front-of-queue Pool, its
  load is split into interleaved quarter descriptors, and its first
  row-tile stores per half, so the store stream starts ~12 us in.

Sharding: 24 (b,h) heads -> 3 per core across 8 cores, full I/O.
"""

import ml_dtypes
import numpy as np

import concourse.mybir as mybir
import concourse.tile as tile
from concourse import bacc
from concourse.bass_utils import run_bass_kernel_spmd

C = [0.17393044, 0.15653739, 0.14088365, 0.12679529, 5.51975209,
     4.96777688, 4.4709992, -1.44776001, -1.30298401, 46.05483778]
MAX_ORDER = 3
X_MAX = 0.9
OFFSET = 0.45

B, H, S, D = 2, 12, 2048, 64
BH = B * H
N_CORES = 8
BLK = BH // N_CORES  # heads per core

M_TILE = 128   # output rows per matmul (PSUM partitions)
N_TILE = 512   # output cols per matmul (one fp32 PSUM bank)

_NC_CACHE = {}
_last_nc = None
_last_in_maps = None


def _coefs():
    cs = []
    idx = 0
    for i in range(MAX_ORDER + 1):
        n_j = MAX_ORDER - i + 1
        cs.append(sum(C[idx + j] * X_MAX ** j for j in range(n_j)))
        idx += n_j
    return cs  # [c0, c1, c2, c3]


def _build_nc(n_chunks, wk):
    """Device kernel: per core, BLK independent [S,S] fp16 output blocks,
    each output tile = sum over n_chunks K=128 bf16 matmuls. Each head's
    a|w operands live in one combined [wk, 2S] DRAM tensor (one load
    descriptor per head; when wk == 64 rows 64:128 are memset to zero so
    the matmul runs at the fast K=128 rate)."""
    nc = bacc.Bacc(None, target_bir_lowering=False)
    aw_d = nc.dram_tensor("aw", [BLK, n_chunks, wk, 2 * S],
                          mybir.dt.bfloat16, kind="ExternalInput")
    out_d = nc.dram_tensor("out", [BLK, S, S], mybir.dt.float16,
                           kind="ExternalOutput")

    with tile.TileContext(nc) as tc:
        with (
            tc.tile_pool(name="inp", bufs=1) as inp,
            tc.tile_pool(name="ps", bufs=4, space="PSUM") as psp,
            tc.tile_pool(name="outp", bufs=14) as outp,
        ):
            # Each head's load is emitted just before its row-tiles so the
            # single DMA FIFO starts storing after only one head's load.
            aw_ts = {}
            for blk in range(BLK):
                for c in range(n_chunks):
                    t = inp.tile([128, 2 * S], mybir.dt.bfloat16,
                                 name=f"aw{blk}_{c}", tag=f"aw{blk}_{c}")
                    aw_ts[(blk, c)] = t
            if wk < 128:
                # head 0 gates the pipeline start: split its zero rows
                # across DVE (idle until the first drains) and the front of
                # the Pool queue so both halves finish in parallel ~8us
                for c in range(n_chunks):
                    nc.vector.memset(aw_ts[(0, c)][wk:, :S], 0.0)
                    nc.gpsimd.memset(aw_ts[(0, c)][wk:, S:], 0.0)
                for blk in range(1, BLK):
                    for c in range(n_chunks):
                        nc.gpsimd.memset(aw_ts[(blk, c)][wk:], 0.0)

            # Pool/GpSimd can't read PSUM on TRN2, so drains go to DVE and
            # Act. A 2-deep ring of [128,2048] PSUM tiles serializes on the
            # ~2.2us whole-tile drain (measured ~2.0us/row-tile cadence);
            # instead use a 4-deep ring of [128,1024] half-tiles whose
            # ~1.1us drains alternate engines.
            HALF = S // 2
            drain_cost = [0.0, 0.0]  # accumulated us on [DVE, Act]
            with nc.allow_low_precision(reason="fp16 out within 2e-2 tol"):
                for blk in range(BLK):
                    for c in range(n_chunks):
                        if blk == 0:
                            # interleaved quarter-loads: the first matmul
                            # needs only the a columns + first w chunk, so
                            # let those land before the rest
                            q = S // 2
                            for part in (0, 2, 1, 3):
                                nc.sync.dma_start(
                                    aw_ts[(blk, c)][:wk,
                                                    part * q:(part + 1) * q],
                                    aw_d[blk, c, :, part * q:(part + 1) * q])
                        else:
                            nc.sync.dma_start(aw_ts[(blk, c)][:wk],
                                              aw_d[blk, c])
                    for st in range(S // M_TILE):
                        ot = outp.tile([M_TILE, S], mybir.dt.float16,
                                       tag="ot")
                        for h in range(2):
                            ps = psp.tile([M_TILE, HALF], mybir.dt.float32,
                                          tag="ps")
                            for ntl in range(HALF // N_TILE):
                                nt = h * (HALF // N_TILE) + ntl
                                for c in range(n_chunks):
                                    nc.tensor.matmul(
                                        ps[:, ntl * N_TILE:
                                           (ntl + 1) * N_TILE],
                                        aw_ts[(blk, c)][
                                            :, st * M_TILE:(st + 1) * M_TILE],
                                        aw_ts[(blk, c)][
                                            :, S + nt * N_TILE:
                                            S + (nt + 1) * N_TILE],
                                        start=(c == 0),
                                        stop=(c == n_chunks - 1),
                                    )
                            dst = ot[:, h * HALF:(h + 1) * HALF]
                            if drain_cost[0] + 1.24 <= drain_cost[1] + 1.03:
                                drain_cost[0] += 1.24
                                nc.vector.tensor_copy(dst, ps[:])
                            else:
                                drain_cost[1] += 1.03
                                nc.scalar.copy(dst, ps[:])
                            if blk == 0 and st == 0:
                                # first row-tile: store each half as soon as
                                # it drains so the store stream starts early
                                nc.sync.dma_start(
                                    out_d[blk, :M_TILE,
                                          h * HALF:(h + 1) * HALF],
                                    dst)
                        if not (blk == 0 and st == 0):
                            nc.sync.dma_start(
                                out_d[blk, st * M_TILE:(st + 1) * M_TILE, :],
                                ot[:])
    nc.compile()
    return nc


def _get_nc(n_chunks, wk):
    key = (n_chunks, wk)
    if key not in _NC_CACHE:
        _NC_CACHE[key] = _build_nc(n_chunks, wk)
    return _NC_CACHE[key]


def _prepare(x, y, dm, qk):
    """Host prep -> bf16 array aw [BH, n_chunks, wk, 2S] (a|w column
    blocks per head)."""
    c0, c1, c2, c3 = _coefs()
    yo = (y - OFFSET).astype(np.float32)  # [B,H,D,S]
    if qk:
        n_chunks, wk = 1, D
        af = np.ascontiguousarray(
            (x * dm[None, None, :, :]).transpose(0, 1, 3, 2)
        ).reshape(BH, 1, D, S).astype(np.float32)
        aw = np.empty((BH, 1, D, 2 * S), dtype=ml_dtypes.bfloat16)
        aw[..., :S] = af
        aw[..., S:] = (((c3 * yo + c2) * yo + c1) * yo + c0) \
            .astype(ml_dtypes.bfloat16).reshape(BH, 1, D, S)
    else:
        n_chunks, wk = 2, 2 * D
        d = dm[:, 0]
        aw = np.empty((BH, 2, 2 * D, 2 * S), dtype=ml_dtypes.bfloat16)
        xt = x.transpose(0, 1, 3, 2).reshape(BH, D, S)
        di = np.ones_like(d)
        yi = np.ones_like(yo).reshape(BH, D, S)
        yo_r = yo.reshape(BH, D, S)
        for i, ci in enumerate((c0, c1, c2, c3)):
            c, half = divmod(i, 2)
            aw[:, c, half * D:(half + 1) * D, :S] = xt * di[None, None, :]
            aw[:, c, half * D:(half + 1) * D, S:] = ci * yi
            di = di * d
            yi = yi * yo_r
    return aw, n_chunks, wk


def kernel(**inputs):
    x = np.asarray(inputs["x"], dtype=np.float32)
    y = np.asarray(inputs["y"], dtype=np.float32)
    dm = np.asarray(inputs["decay_mask"], dtype=np.float32)
    qk = int(np.asarray(inputs["QK_mul"]))

    aw, n_chunks, wk = _prepare(x, y, dm, qk)
    nc = _get_nc(n_chunks, wk)

    in_maps = [
        {"aw": aw[c * BLK:(c + 1) * BLK]} for c in range(N_CORES)
    ]
    global _last_nc, _last_in_maps
    _last_nc, _last_in_maps = nc, in_maps

    res = None
    for attempt in range(3):
        try:
            res = run_bass_kernel_spmd(nc, in_maps,
                                       core_ids=list(range(N_CORES)))
            break
        except Exception:
            # transient NRT_EXEC_UNIT_UNRECOVERABLE wedges occur on busy axon
            # terminals; they clear after a pause
            if attempt == 2:
                raise
            import time
            time.sleep(45)

    out = np.empty((BH, S, S), dtype=np.float32)
    for c in range(N_CORES):
        out[c * BLK:(c + 1) * BLK] = res.results[c]["out"]
    return out.reshape(B, H, S, S)


# revision 38
# speedup vs baseline: 1.0187x; 1.0187x over previous
"""Trainium2 Bass kernel for nn_DRAM_MAC_temporal_encoding (polynomial attention).

Math (QK_mul=1):
    out = sum_i coef_i * (x @ (y-OFF)^i) * decay
        = (x * decay) @ P(y-OFF)            # P = Horner cubic, elementwise
so the whole problem is ONE [S,64]@[64,S] matmul per (b,h) head plus the
output write -> memory-bound. The tiny elementwise prep (poly on y,
row-scaling x, transpose, bf16 cast) runs on host; the device does
matmuls + store. QK_mul=0: out = sum_i coef_i * ((x*d^i) @ (y-OFF)^i) ->
two K=128 chunks (4 stacked K=64 terms), same kernel with n_chunks=2.

Perf model (per core: 3 heads, 25.2 MiB of output):
- Tolerance is rel_err < 2e-2, so inputs are plain bf16 (one matmul per
  output tile, no hi/lo split; measured 1.2e-3) and the output is stored
  as fp16 (half the fp32 store traffic; host upcasts). DMA is the
  bottleneck: 16 engines x ~25.6 GB/s = ~410 GB/s per core, ~63 us of
  stores, and in clean windows the engines run at 99-101% utilization.
- K=64 matmuls stream at ~1/3 the K=128 rate on HW (630 vs 233 ns per
  [128,512]), so K is presented as 128 with zero rows 64:128 (memset once;
  zero rows add no HBM traffic).
- PSUM->SBUF drains (fp32->fp16) are the only engines that can read PSUM
  (DVE + Act; Pool cannot): [128,1024] half-tiles on a 4-deep PSUM ring,
  greedily balanced across the two engines. A 2-deep [128,2048] ring
  serialized PE fill against the 2.2 us whole-tile drain.
- Startup: head 0's zero-row memsets go to DVE + front-of-queue Pool, its
  load is split into interleaved quarter descriptors, and its first
  row-tile stores per half, so the store stream starts ~12 us in.

Sharding: 24 (b,h) heads -> 3 per core across 8 cores, full I/O.
"""

import ml_dtypes
import numpy as np

import concourse.mybir as mybir
import concourse.tile as tile
from concourse import bacc
from concourse.bass_utils import run_bass_kernel_spmd

C = [0.17393044, 0.15653739, 0.14088365, 0.12679529, 5.51975209,
     4.96777688, 4.4709992, -1.44776001, -1.30298401, 46.05483778]
MAX_ORDER = 3
X_MAX = 0.9
OFFSET = 0.45

B, H, S, D = 2, 12, 2048, 64
BH = B * H
N_CORES = 8
BLK = BH // N_CORES  # heads per core

M_TILE = 128   # output rows per matmul (PSUM partitions)
N_TILE = 512   # output cols per matmul (one fp32 PSUM bank)

_NC_CACHE = {}
_last_nc = None
_last_in_maps = None


def _coefs():
    cs = []
    idx = 0
    for i in range(MAX_ORDER + 1):
        n_j = MAX_ORDER - i + 1
        cs.append(sum(C[idx + j] * X_MAX ** j for j in range(n_j)))
        idx += n_j
    return cs  # [c0, c1, c2, c3]


def _build_nc(n_chunks, wk):
    """Device kernel: per core, BLK independent [S,S] fp16 output blocks,
    each output tile = sum over n_chunks K=128 bf16 matmuls. Each head's
    a|w operands live in one combined [wk, 2S] DRAM tensor (one load
    descriptor per head; when wk == 64 rows 64:128 are memset to zero so
    the matmul runs at the fast K=128 rate)."""
    nc = bacc.Bacc(None, target_bir_lowering=False)
    aw_d = nc.dram_tensor("aw", [BLK, n_chunks, wk, 2 * S],
                          mybir.dt.bfloat16, kind="ExternalInput")
    out_d = nc.dram_tensor("out", [BLK, S, S], mybir.dt.float16,
                           kind="ExternalOutput")

    with tile.TileContext(nc) as tc:
        with (
            tc.tile_pool(name="inp", bufs=1) as inp,
            tc.tile_pool(name="ps", bufs=4, space="PSUM") as psp,
            tc.tile_pool(name="outp", bufs=14) as outp,
        ):
            # Each head's load is emitted just before its row-tiles so the
            # single DMA FIFO starts storing after only one head's load.
            aw_ts = {}
            for blk in range(BLK):
                for c in range(n_chunks):
                    t = inp.tile([128, 2 * S], mybir.dt.bfloat16,
                                 name=f"aw{blk}_{c}", tag=f"aw{blk}_{c}")
                    aw_ts[(blk, c)] = t
            if wk < 128:
                # head 0 gates the pipeline start: split its zero rows
                # across DVE (idle until the first drains) and the front of
                # the Pool queue so both halves finish in parallel ~8us
                for c in range(n_chunks):
                    nc.vector.memset(aw_ts[(0, c)][wk:, :S], 0.0)
                    nc.gpsimd.memset(aw_ts[(0, c)][wk:, S:], 0.0)
                for blk in range(1, BLK):
                    for c in range(n_chunks):
                        nc.gpsimd.memset(aw_ts[(blk, c)][wk:], 0.0)

            # Pool/GpSimd can't read PSUM on TRN2, so drains go to DVE and
            # Act. A 2-deep ring of [128,2048] PSUM tiles serializes on the
            # ~2.2us whole-tile drain (measured ~2.0us/row-tile cadence);
            # instead use a 4-deep ring of [128,1024] half-tiles whose
            # ~1.1us drains alternate engines.
            # All loads are issued up front, head 0's first as interleaved
            # quarter-descriptors (its first matmul needs only the a columns
            # + first w chunk). Heads 1-2's loads then fill the DMA's
            # otherwise-idle pipeline-fill bubble instead of interrupting
            # the saturated store stream mid-run.
            for blk in range(BLK):
                for c in range(n_chunks):
                    if blk == 0:
                        q = S // 2
                        for part in (0, 2, 1, 3):
                            nc.sync.dma_start(
                                aw_ts[(blk, c)][:wk,
                                                part * q:(part + 1) * q],
                                aw_d[blk, c, :, part * q:(part + 1) * q])
                    else:
                        nc.sync.dma_start(aw_ts[(blk, c)][:wk],
                                          aw_d[blk, c])

            HALF = S // 2
            drain_cost = [0.0, 0.0]  # accumulated us on [DVE, Act]
            with nc.allow_low_precision(reason="fp16 out within 2e-2 tol"):
                for blk in range(BLK):
                    for st in range(S // M_TILE):
                        ot = outp.tile([M_TILE, S], mybir.dt.float16,
                                       tag="ot")
                        for h in range(2):
                            ps = psp.tile([M_TILE, HALF], mybir.dt.float32,
                                          tag="ps")
                            for ntl in range(HALF // N_TILE):
                                nt = h * (HALF // N_TILE) + ntl
                                for c in range(n_chunks):
                                    nc.tensor.matmul(
                                        ps[:, ntl * N_TILE:
                                           (ntl + 1) * N_TILE],
                                        aw_ts[(blk, c)][
                                            :, st * M_TILE:(st + 1) * M_TILE],
                                        aw_ts[(blk, c)][
                                            :, S + nt * N_TILE:
                                            S + (nt + 1) * N_TILE],
                                        start=(c == 0),
                                        stop=(c == n_chunks - 1),
                                    )
                            dst = ot[:, h * HALF:(h + 1) * HALF]
                            if drain_cost[0] + 1.24 <= drain_cost[1] + 1.03:
                                drain_cost[0] += 1.24
                                nc.vector.tensor_copy(dst, ps[:])
                            else:
                                drain_cost[1] += 1.03
                                nc.scalar.copy(dst, ps[:])
                            if blk == 0 and st == 0:
                                # first row-tile: store each half as soon as
                                # it drains so the store stream starts early
                                nc.sync.dma_start(
                                    out_d[blk, :M_TILE,
                                          h * HALF:(h + 1) * HALF],
                                    dst)
                        if not (blk == 0 and st == 0):
                            nc.sync.dma_start(
                                out_d[blk, st * M_TILE:(st + 1) * M_TILE, :],
                                ot[:])
    nc.compile()
    return nc


def _get_nc(n_chunks, wk):
    key = (n_chunks, wk)
    if key not in _NC_CACHE:
        _NC_CACHE[key] = _build_nc(n_chunks, wk)
    return _NC_CACHE[key]


def _prepare(x, y, dm, qk):
    """Host prep -> bf16 array aw [BH, n_chunks, wk, 2S] (a|w column
    blocks per head)."""
    c0, c1, c2, c3 = _coefs()
    yo = (y - OFFSET).astype(np.float32)  # [B,H,D,S]
    if qk:
        n_chunks, wk = 1, D
        af = np.ascontiguousarray(
            (x * dm[None, None, :, :]).transpose(0, 1, 3, 2)
        ).reshape(BH, 1, D, S).astype(np.float32)
        aw = np.empty((BH, 1, D, 2 * S), dtype=ml_dtypes.bfloat16)
        aw[..., :S] = af
        aw[..., S:] = (((c3 * yo + c2) * yo + c1) * yo + c0) \
            .astype(ml_dtypes.bfloat16).reshape(BH, 1, D, S)
    else:
        n_chunks, wk = 2, 2 * D
        d = dm[:, 0]
        aw = np.empty((BH, 2, 2 * D, 2 * S), dtype=ml_dtypes.bfloat16)
        xt = x.transpose(0, 1, 3, 2).reshape(BH, D, S)
        di = np.ones_like(d)
        yi = np.ones_like(yo).reshape(BH, D, S)
        yo_r = yo.reshape(BH, D, S)
        for i, ci in enumerate((c0, c1, c2, c3)):
            c, half = divmod(i, 2)
            aw[:, c, half * D:(half + 1) * D, :S] = xt * di[None, None, :]
            aw[:, c, half * D:(half + 1) * D, S:] = ci * yi
            di = di * d
            yi = yi * yo_r
    return aw, n_chunks, wk


def kernel(**inputs):
    x = np.asarray(inputs["x"], dtype=np.float32)
    y = np.asarray(inputs["y"], dtype=np.float32)
    dm = np.asarray(inputs["decay_mask"], dtype=np.float32)
    qk = int(np.asarray(inputs["QK_mul"]))

    aw, n_chunks, wk = _prepare(x, y, dm, qk)
    nc = _get_nc(n_chunks, wk)

    in_maps = [
        {"aw": aw[c * BLK:(c + 1) * BLK]} for c in range(N_CORES)
    ]
    global _last_nc, _last_in_maps
    _last_nc, _last_in_maps = nc, in_maps

    res = None
    for attempt in range(3):
        try:
            res = run_bass_kernel_spmd(nc, in_maps,
                                       core_ids=list(range(N_CORES)))
            break
        except Exception:
            # transient NRT_EXEC_UNIT_UNRECOVERABLE wedges occur on busy axon
            # terminals; they clear after a pause
            if attempt == 2:
                raise
            import time
            time.sleep(45)

    out = np.empty((BH, S, S), dtype=np.float32)
    for c in range(N_CORES):
        out[c * BLK:(c + 1) * BLK] = res.results[c]["out"]
    return out.reshape(B, H, S, S)
